# revision 9
# baseline (speedup 1.0000x reference)
"""DynamicBalanceLoss on 8 Trainium2 NeuronCores.

Math (per sample i, C=32000 classes):
    losses[i] = logsumexp(x[i]) - x[i, target[i]]
    pred[i]   = argmax(x[i])
    in dict e (true,pred,w): (1,0,2.0),(2,1,1.5),(0,3,3.0)
    loss1_sum = sum_{match} losses*w ; loss2_sum = sum_{no match} losses
Final scalar combine happens on host from per-core [128,3] partials.

Sharding: data-parallel over batch, 1024 rows/core. Each core streams its
[1024, 32000] f32 shard once from HBM (memory-bound): per [128, 8000] tile a
DVE max-reduce and an in-place ACT exp with fused per-partition sum
(online logsumexp). argmax is never computed: pred==p iff x[:,p]==rowmax
(exact in f32; ties have measure ~0 for randn inputs).
"""

import os
import sys

sys.path.insert(0, "/opt/trn_rl_repo")

import numpy as np

import concourse.bacc as bacc
import concourse.bass as bass
import concourse.mybir as mybir
import concourse.tile as tile
from concourse.bass_utils import run_bass_kernel_spmd

# Problem constants (hardcoded per contract)
B, C = 8192, 32000
NCORES = 8
B_LOC = B // NCORES          # 1024 rows per core
P = 128                      # partitions
RB = B_LOC // P              # 8 row blocks per core
W = 8000                     # column tile width
NCT = C // W                 # 4 column tiles
K_HYP = 0.5
T_HYP = 3.0
EW_TRUE = (1, 2, 0)
EW_PRED = (0, 1, 3)          # all < W, so they live in column-tile 0
EW_W = (2.0, 1.5, 3.0)

F32 = mybir.dt.float32
I32 = mybir.dt.int32
AX = mybir.AxisListType
OP = mybir.AluOpType
AF = mybir.ActivationFunctionType

_CACHE = {}

# device-side indirect-DMA gather of target logits; default is the
# host-gathered input path
USE_INDIRECT = os.environ.get("DBL_USE_INDIRECT", "0") == "1"


def _build():
    if "nc" in _CACHE:
        return _CACHE["nc"]
    nc = bacc.Bacc("TRN2", target_bir_lowering=False, debug=False)
    x = nc.dram_tensor("x", [B_LOC, C], F32, kind="ExternalInput")
    tgt = nc.dram_tensor("tgt", [P, RB], F32, kind="ExternalInput")
    if USE_INDIRECT:
        toff = nc.dram_tensor("toff", [P, RB], I32, kind="ExternalInput")
    else:
        tlog = nc.dram_tensor("tlog", [P, RB], F32, kind="ExternalInput")
    partials = nc.dram_tensor("partials", [P, 3], F32, kind="ExternalOutput")

    with tile.TileContext(nc) as tc:
        with (
            tc.tile_pool(name="big", bufs=4) as bigp,
            tc.tile_pool(name="small", bufs=1) as smp,
        ):
            tgt_s = smp.tile([P, RB], F32, tag="tgt_s")
            nc.sync.dma_start(tgt_s[:], tgt[:])

            tl = smp.tile([P, RB], F32, tag="tl")
            if USE_INDIRECT:
                # gather target logits: tl[p, j] = x.flat[toff[p, j]]
                toff_s = smp.tile([P, RB], I32, tag="toff_s")
                nc.sync.dma_start(toff_s[:], toff[:])
                nc.gpsimd.indirect_dma_start(
                    out=tl[:],
                    out_offset=None,
                    in_=x[:, :].rearrange("a (b o) -> (a b) o", o=1),
                    in_offset=bass.IndirectOffsetOnAxis(ap=toff_s[:], axis=0),
                )
            else:
                nc.sync.dma_start(tl[:], tlog[:])

            mall = smp.tile([P, RB * NCT], F32, tag="mall")    # +max per tile
            nmall = smp.tile([P, RB * NCT], F32, tag="nmall")  # -max per tile
            sall = smp.tile([P, RB * NCT], F32, tag="sall")    # sum exp per tile
            c013 = smp.tile([P, 3 * RB], F32, tag="c013")      # x cols {0,1,3} per j

            # main streaming pass
            for j in range(RB):
                for c in range(NCT):
                    k = j * NCT + c
                    xt = bigp.tile([P, W], F32, tag="xt")
                    nc.sync.dma_start(
                        xt[:], x[j * P:(j + 1) * P, c * W:(c + 1) * W]
                    )
                    nc.vector.tensor_reduce(
                        out=mall[:, k:k + 1], in_=xt[:], axis=AX.X, op=OP.max,
                    )
                    nc.vector.tensor_scalar(
                        out=nmall[:, k:k + 1], in0=mall[:, k:k + 1],
                        scalar1=-1.0, scalar2=None, op0=OP.mult,
                    )
                    if c == 0:
                        nc.vector.tensor_copy(
                            out=c013[:, 3 * j:3 * j + 2], in_=xt[:, 0:2]
                        )
                        nc.vector.tensor_copy(
                            out=c013[:, 3 * j + 2:3 * j + 3], in_=xt[:, 3:4]
                        )
                    # in-place exp(x - m) with fused per-partition sum
                    nc.scalar.activation(
                        out=xt[:], in_=xt[:], func=AF.Exp,
                        bias=nmall[:, k:k + 1], scale=1.0,
                        accum_out=sall[:, k:k + 1],
                    )

            # ---- epilogue (all small ops on [P, *] 2-D tiles) ----
            mrow = smp.tile([P, RB], F32, tag="mrow")   # +rowmax
            nmax = smp.tile([P, RB], F32, tag="nmax")   # -rowmax
            for j in range(RB):
                nc.vector.tensor_reduce(
                    out=mrow[:, j:j + 1], in_=mall[:, j * NCT:(j + 1) * NCT],
                    axis=AX.X, op=OP.max,
                )
            nc.vector.tensor_scalar(
                out=nmax[:], in0=mrow[:], scalar1=-1.0, scalar2=None, op0=OP.mult
            )

            # corr = exp(m_jc - M_j): diff = nmall - nmax -> exp(-diff)
            diff = smp.tile([P, RB * NCT], F32, tag="diff")
            for j in range(RB):
                nc.vector.tensor_scalar(
                    out=diff[:, j * NCT:(j + 1) * NCT],
                    in0=nmall[:, j * NCT:(j + 1) * NCT],
                    scalar1=nmax[:, j:j + 1], scalar2=None, op0=OP.subtract,
                )
            nc.scalar.activation(out=diff[:], in_=diff[:], func=AF.Exp, scale=-1.0)
            nc.vector.tensor_tensor(out=sall[:], in0=sall[:], in1=diff[:], op=OP.mult)
            S = smp.tile([P, RB], F32, tag="S")
            for j in range(RB):
                nc.vector.tensor_reduce(
                    out=S[:, j:j + 1], in_=sall[:, j * NCT:(j + 1) * NCT],
                    axis=AX.X, op=OP.add,
                )

            logS = smp.tile([P, RB], F32, tag="logS")
            nc.scalar.activation(out=logS[:], in_=S[:], func=AF.Ln)

            # losses = logS - nmax - tl  (= M + logS - tl)
            losses = smp.tile([P, RB], F32, tag="losses")
            nc.vector.tensor_tensor(out=losses[:], in0=logS[:], in1=nmax[:], op=OP.subtract)
            nc.vector.tensor_tensor(out=losses[:], in0=losses[:], in1=tl[:], op=OP.subtract)

            # pred-match: x[:,p]==M for p in {0,1,3}; target-match vs {1,2,0}
            pm = smp.tile([P, 3 * RB], F32, tag="pm")
            tm = smp.tile([P, 3 * RB], F32, tag="tm")
            for j in range(RB):
                nc.vector.tensor_scalar(
                    out=pm[:, 3 * j:3 * j + 3], in0=c013[:, 3 * j:3 * j + 3],
                    scalar1=mrow[:, j:j + 1], scalar2=None, op0=OP.is_equal,
                )
                for e in range(3):
                    nc.vector.tensor_scalar(
                        out=tm[:, 3 * j + e:3 * j + e + 1],
                        in0=tgt_s[:, j:j + 1],
                        scalar1=float(EW_TRUE[e]), scalar2=None, op0=OP.is_equal,
                    )
            nc.vector.tensor_tensor(out=pm[:], in0=pm[:], in1=tm[:], op=OP.mult)

            # w[p,j] = sum_e match*EW_W[e]; at most one entry matches
            eww = smp.tile([P, 3], F32, tag="eww")
            for e in range(3):
                nc.vector.memset(eww[:, e:e + 1], float(EW_W[e]))
            wts = smp.tile([P, RB], F32, tag="wts")
            wm = smp.tile([P, 3], F32, tag="wm")
            for j in range(RB):
                nc.vector.tensor_tensor(
                    out=wm[:], in0=pm[:, 3 * j:3 * j + 3], in1=eww[:], op=OP.mult
                )
                nc.vector.tensor_reduce(
                    out=wts[:, j:j + 1], in_=wm[:], axis=AX.X, op=OP.add
                )
            ind = smp.tile([P, RB], F32, tag="ind")
            nc.vector.tensor_scalar(
                out=ind[:], in0=wts[:], scalar1=0.0, scalar2=None, op0=OP.is_gt
            )

            q0 = smp.tile([P, RB], F32, tag="q0")
            nc.vector.tensor_tensor(out=q0[:], in0=losses[:], in1=wts[:], op=OP.mult)
            # q1 = losses*(1-ind) = losses - losses*ind
            li = smp.tile([P, RB], F32, tag="li")
            nc.vector.tensor_tensor(out=li[:], in0=losses[:], in1=ind[:], op=OP.mult)
            q1 = smp.tile([P, RB], F32, tag="q1")
            nc.vector.tensor_tensor(out=q1[:], in0=losses[:], in1=li[:], op=OP.subtract)

            res = smp.tile([P, 3], F32, tag="res")
            nc.vector.tensor_reduce(out=res[:, 0:1], in_=q0[:], axis=AX.X, op=OP.add)
            nc.vector.tensor_reduce(out=res[:, 1:2], in_=q1[:], axis=AX.X, op=OP.add)
            nc.vector.tensor_reduce(out=res[:, 2:3], in_=ind[:], axis=AX.X, op=OP.add)
            nc.sync.dma_start(partials[:], res[:])

    nc.compile()
    _CACHE["nc"] = nc
    return nc


def make_in_maps(output, target):
    output = np.ascontiguousarray(np.asarray(output, dtype=np.float32))
    target = np.asarray(target).astype(np.int64)
    in_maps = []
    for k in range(NCORES):
        rows = slice(k * B_LOC, (k + 1) * B_LOC)
        t = target[rows]
        xk = np.ascontiguousarray(output[rows])
        tgt = np.ascontiguousarray(t.astype(np.float32).reshape(RB, P).T)
        m = {"x": xk, "tgt": tgt}
        if USE_INDIRECT:
            m["toff"] = np.ascontiguousarray(
                (np.arange(B_LOC, dtype=np.int64) * C + t).astype(np.int32)
                .reshape(RB, P).T
            )
        else:
            m["tlog"] = np.ascontiguousarray(
                xk[np.arange(B_LOC), t].astype(np.float32).reshape(RB, P).T
            )
        in_maps.append(m)
    return in_maps


def combine(partials_list):
    """Host-side final scalar combine from per-core [128,3] partials."""
    loss1_sum = 0.0
    loss2_sum = 0.0
    count1 = 0.0
    for pt in partials_list:
        pt = np.asarray(pt, dtype=np.float64)
        loss1_sum += pt[:, 0].sum()
        loss2_sum += pt[:, 1].sum()
        count1 += pt[:, 2].sum()
    count1 = int(round(count1))
    count2 = B - count1
    loss1 = loss1_sum / B if count1 > 0 else loss1_sum
    loss2 = loss2_sum / B if count2 > 0 else loss2_sum
    if loss1 > 0:
        w_l2 = 1.0 / (1.0 + np.exp(-K_HYP * (loss1 - T_HYP)))
        total = loss1 + w_l2 * loss2
    else:
        total = loss2 / B
    return np.float32(total)


def _check_ew(ew_true, ew_pred, ew_w):
    if ew_true is None:
        return
    ok = (
        tuple(np.asarray(ew_true).tolist()) == EW_TRUE
        and tuple(np.asarray(ew_pred).tolist()) == EW_PRED
        and tuple(np.asarray(ew_w, dtype=np.float64).tolist()) == EW_W
    )
    if not ok:
        raise ValueError("error_weights table differs from compiled-in constants")


def run(output, target, trace=False, **trace_kwargs):
    nc = _build()
    in_maps = make_in_maps(output, target)
    br = run_bass_kernel_spmd(
        nc, in_maps, list(range(NCORES)), trace=trace, **trace_kwargs
    )
    total = combine([r["partials"] for r in br.results])
    return total, br


def kernel(output, target, ew_true=None, ew_pred=None, ew_w=None):
    _check_ew(ew_true, ew_pred, ew_w)
    total, _ = run(output, target, trace=False)
    return total


# revision 10
# speedup vs baseline: 1.1524x; 1.1524x over previous
"""DynamicBalanceLoss on 8 Trainium2 NeuronCores.

Math (per sample i, C=32000 classes):
    losses[i] = logsumexp(x[i]) - x[i, target[i]]
    pred[i]   = argmax(x[i])
    in dict e (true,pred,w): (1,0,2.0),(2,1,1.5),(0,3,3.0)
    loss1_sum = sum_{match} losses*w ; loss2_sum = sum_{no match} losses
Final scalar combine happens on host from per-core [128,3] partials.

Sharding: data-parallel over batch, 1024 rows/core. Each core streams its
[1024, 32000] f32 shard once from HBM (memory-bound): per [128, 8000] tile a
DVE max-reduce and an in-place ACT exp with fused per-partition sum
(online logsumexp). argmax is never computed: pred==p iff x[:,p]==rowmax
(exact in f32; ties have measure ~0 for randn inputs).
"""

import os
import sys

sys.path.insert(0, "/opt/trn_rl_repo")

import numpy as np

import concourse.bacc as bacc
import concourse.bass as bass
import concourse.mybir as mybir
import concourse.tile as tile
from concourse.bass_utils import run_bass_kernel_spmd

# Problem constants (hardcoded per contract)
B, C = 8192, 32000
NCORES = 8
B_LOC = B // NCORES          # 1024 rows per core
P = 128                      # partitions
RB = B_LOC // P              # 8 row blocks per core
W = 8000                     # column tile width
NCT = C // W                 # 4 column tiles
K_HYP = 0.5
T_HYP = 3.0
EW_TRUE = (1, 2, 0)
EW_PRED = (0, 1, 3)          # all < W, so they live in column-tile 0
EW_W = (2.0, 1.5, 3.0)

F32 = mybir.dt.float32
I32 = mybir.dt.int32
AX = mybir.AxisListType
OP = mybir.AluOpType
AF = mybir.ActivationFunctionType

_CACHE = {}

# device-side indirect-DMA gather of target logits; default is the
# host-gathered input path
USE_INDIRECT = os.environ.get("DBL_USE_INDIRECT", "0") == "1"


def _build():
    if "nc" in _CACHE:
        return _CACHE["nc"]
    nc = bacc.Bacc("TRN2", target_bir_lowering=False, debug=False)
    x = nc.dram_tensor("x", [B_LOC, C], F32, kind="ExternalInput")
    tgt = nc.dram_tensor("tgt", [P, RB], F32, kind="ExternalInput")
    if USE_INDIRECT:
        toff = nc.dram_tensor("toff", [P, RB], I32, kind="ExternalInput")
    else:
        tlog = nc.dram_tensor("tlog", [P, RB], F32, kind="ExternalInput")
    partials = nc.dram_tensor("partials", [P, 3], F32, kind="ExternalOutput")

    with tile.TileContext(nc) as tc:
        with (
            tc.tile_pool(name="big", bufs=5) as bigp,
            tc.tile_pool(name="small", bufs=1) as smp,
        ):
            tgt_s = smp.tile([P, RB], F32, tag="tgt_s")
            nc.sync.dma_start(tgt_s[:], tgt[:])

            tl = smp.tile([P, RB], F32, tag="tl")
            if USE_INDIRECT:
                # gather target logits: tl[p, j] = x.flat[toff[p, j]]
                toff_s = smp.tile([P, RB], I32, tag="toff_s")
                nc.sync.dma_start(toff_s[:], toff[:])
                nc.gpsimd.indirect_dma_start(
                    out=tl[:],
                    out_offset=None,
                    in_=x[:, :].rearrange("a (b o) -> (a b) o", o=1),
                    in_offset=bass.IndirectOffsetOnAxis(ap=toff_s[:], axis=0),
                )
            else:
                nc.sync.dma_start(tl[:], tlog[:])

            mall = smp.tile([P, RB * NCT], F32, tag="mall")    # +max per tile
            nmall = smp.tile([P, RB * NCT], F32, tag="nmall")  # -max per tile
            sall = smp.tile([P, RB * NCT], F32, tag="sall")    # sum exp per tile
            c013 = smp.tile([P, 3 * RB], F32, tag="c013")      # x cols {0,1,3} per j

            # main streaming pass
            for j in range(RB):
                for c in range(NCT):
                    k = j * NCT + c
                    xt = bigp.tile([P, W], F32, tag="xt")
                    nc.sync.dma_start(
                        xt[:], x[j * P:(j + 1) * P, c * W:(c + 1) * W]
                    )
                    nc.vector.tensor_reduce(
                        out=mall[:, k:k + 1], in_=xt[:], axis=AX.X, op=OP.max,
                    )
                    nc.vector.tensor_scalar(
                        out=nmall[:, k:k + 1], in0=mall[:, k:k + 1],
                        scalar1=-1.0, scalar2=None, op0=OP.mult,
                    )
                    if c == 0:
                        nc.vector.tensor_copy(
                            out=c013[:, 3 * j:3 * j + 2], in_=xt[:, 0:2]
                        )
                        nc.vector.tensor_copy(
                            out=c013[:, 3 * j + 2:3 * j + 3], in_=xt[:, 3:4]
                        )
                    # in-place exp(x - m) with fused per-partition sum
                    nc.scalar.activation(
                        out=xt[:], in_=xt[:], func=AF.Exp,
                        bias=nmall[:, k:k + 1], scale=1.0,
                        accum_out=sall[:, k:k + 1],
                    )

            # ---- epilogue (all small ops on [P, *] 2-D tiles) ----
            mrow = smp.tile([P, RB], F32, tag="mrow")   # +rowmax
            nmax = smp.tile([P, RB], F32, tag="nmax")   # -rowmax
            for j in range(RB):
                nc.vector.tensor_reduce(
                    out=mrow[:, j:j + 1], in_=mall[:, j * NCT:(j + 1) * NCT],
                    axis=AX.X, op=OP.max,
                )
            nc.vector.tensor_scalar(
                out=nmax[:], in0=mrow[:], scalar1=-1.0, scalar2=None, op0=OP.mult
            )

            # corr = exp(m_jc - M_j): diff = nmall - nmax -> exp(-diff)
            diff = smp.tile([P, RB * NCT], F32, tag="diff")
            for j in range(RB):
                nc.vector.tensor_scalar(
                    out=diff[:, j * NCT:(j + 1) * NCT],
                    in0=nmall[:, j * NCT:(j + 1) * NCT],
                    scalar1=nmax[:, j:j + 1], scalar2=None, op0=OP.subtract,
                )
            nc.scalar.activation(out=diff[:], in_=diff[:], func=AF.Exp, scale=-1.0)
            nc.vector.tensor_tensor(out=sall[:], in0=sall[:], in1=diff[:], op=OP.mult)
            S = smp.tile([P, RB], F32, tag="S")
            for j in range(RB):
                nc.vector.tensor_reduce(
                    out=S[:, j:j + 1], in_=sall[:, j * NCT:(j + 1) * NCT],
                    axis=AX.X, op=OP.add,
                )

            logS = smp.tile([P, RB], F32, tag="logS")
            nc.scalar.activation(out=logS[:], in_=S[:], func=AF.Ln)

            # losses = logS - nmax - tl  (= M + logS - tl)
            losses = smp.tile([P, RB], F32, tag="losses")
            nc.vector.tensor_tensor(out=losses[:], in0=logS[:], in1=nmax[:], op=OP.subtract)
            nc.vector.tensor_tensor(out=losses[:], in0=losses[:], in1=tl[:], op=OP.subtract)

            # pred-match: x[:,p]==M for p in {0,1,3}; target-match vs {1,2,0}
            pm = smp.tile([P, 3 * RB], F32, tag="pm")
            tm = smp.tile([P, 3 * RB], F32, tag="tm")
            for j in range(RB):
                nc.vector.tensor_scalar(
                    out=pm[:, 3 * j:3 * j + 3], in0=c013[:, 3 * j:3 * j + 3],
                    scalar1=mrow[:, j:j + 1], scalar2=None, op0=OP.is_equal,
                )
                for e in range(3):
                    nc.vector.tensor_scalar(
                        out=tm[:, 3 * j + e:3 * j + e + 1],
                        in0=tgt_s[:, j:j + 1],
                        scalar1=float(EW_TRUE[e]), scalar2=None, op0=OP.is_equal,
                    )
            nc.vector.tensor_tensor(out=pm[:], in0=pm[:], in1=tm[:], op=OP.mult)

            # w[p,j] = sum_e match*EW_W[e]; at most one entry matches
            eww = smp.tile([P, 3], F32, tag="eww")
            for e in range(3):
                nc.vector.memset(eww[:, e:e + 1], float(EW_W[e]))
            wts = smp.tile([P, RB], F32, tag="wts")
            wm = smp.tile([P, 3], F32, tag="wm")
            for j in range(RB):
                nc.vector.tensor_tensor(
                    out=wm[:], in0=pm[:, 3 * j:3 * j + 3], in1=eww[:], op=OP.mult
                )
                nc.vector.tensor_reduce(
                    out=wts[:, j:j + 1], in_=wm[:], axis=AX.X, op=OP.add
                )
            ind = smp.tile([P, RB], F32, tag="ind")
            nc.vector.tensor_scalar(
                out=ind[:], in0=wts[:], scalar1=0.0, scalar2=None, op0=OP.is_gt
            )

            q0 = smp.tile([P, RB], F32, tag="q0")
            nc.vector.tensor_tensor(out=q0[:], in0=losses[:], in1=wts[:], op=OP.mult)
            # q1 = losses*(1-ind) = losses - losses*ind
            li = smp.tile([P, RB], F32, tag="li")
            nc.vector.tensor_tensor(out=li[:], in0=losses[:], in1=ind[:], op=OP.mult)
            q1 = smp.tile([P, RB], F32, tag="q1")
            nc.vector.tensor_tensor(out=q1[:], in0=losses[:], in1=li[:], op=OP.subtract)

            res = smp.tile([P, 3], F32, tag="res")
            nc.vector.tensor_reduce(out=res[:, 0:1], in_=q0[:], axis=AX.X, op=OP.add)
            nc.vector.tensor_reduce(out=res[:, 1:2], in_=q1[:], axis=AX.X, op=OP.add)
            nc.vector.tensor_reduce(out=res[:, 2:3], in_=ind[:], axis=AX.X, op=OP.add)
            nc.sync.dma_start(partials[:], res[:])

    nc.compile()
    _CACHE["nc"] = nc
    return nc


def make_in_maps(output, target):
    output = np.ascontiguousarray(np.asarray(output, dtype=np.float32))
    target = np.asarray(target).astype(np.int64)
    in_maps = []
    for k in range(NCORES):
        rows = slice(k * B_LOC, (k + 1) * B_LOC)
        t = target[rows]
        xk = np.ascontiguousarray(output[rows])
        tgt = np.ascontiguousarray(t.astype(np.float32).reshape(RB, P).T)
        m = {"x": xk, "tgt": tgt}
        if USE_INDIRECT:
            m["toff"] = np.ascontiguousarray(
                (np.arange(B_LOC, dtype=np.int64) * C + t).astype(np.int32)
                .reshape(RB, P).T
            )
        else:
            m["tlog"] = np.ascontiguousarray(
                xk[np.arange(B_LOC), t].astype(np.float32).reshape(RB, P).T
            )
        in_maps.append(m)
    return in_maps


def combine(partials_list):
    """Host-side final scalar combine from per-core [128,3] partials."""
    loss1_sum = 0.0
    loss2_sum = 0.0
    count1 = 0.0
    for pt in partials_list:
        pt = np.asarray(pt, dtype=np.float64)
        loss1_sum += pt[:, 0].sum()
        loss2_sum += pt[:, 1].sum()
        count1 += pt[:, 2].sum()
    count1 = int(round(count1))
    count2 = B - count1
    loss1 = loss1_sum / B if count1 > 0 else loss1_sum
    loss2 = loss2_sum / B if count2 > 0 else loss2_sum
    if loss1 > 0:
        w_l2 = 1.0 / (1.0 + np.exp(-K_HYP * (loss1 - T_HYP)))
        total = loss1 + w_l2 * loss2
    else:
        total = loss2 / B
    return np.float32(total)


def _check_ew(ew_true, ew_pred, ew_w):
    if ew_true is None:
        return
    ok = (
        tuple(np.asarray(ew_true).tolist()) == EW_TRUE
        and tuple(np.asarray(ew_pred).tolist()) == EW_PRED
        and tuple(np.asarray(ew_w, dtype=np.float64).tolist()) == EW_W
    )
    if not ok:
        raise ValueError("error_weights table differs from compiled-in constants")


def run(output, target, trace=False, **trace_kwargs):
    nc = _build()
    in_maps = make_in_maps(output, target)
    br = run_bass_kernel_spmd(
        nc, in_maps, list(range(NCORES)), trace=trace, **trace_kwargs
    )
    total = combine([r["partials"] for r in br.results])
    return total, br


def kernel(output, target, ew_true=None, ew_pred=None, ew_w=None):
    _check_ew(ew_true, ew_pred, ew_w)
    total, _ = run(output, target, trace=False)
    return total
